# revision 70
# baseline (speedup 1.0000x reference)
"""Banded multi-headed attention (nn_BandedMultiheadedAttention) on 8 Trainium2 NeuronCores.

Sharding: data-parallel over (batch, sequence-chunk): core c handles batch c//4,
query positions [256*(c%4), 256*(c%4)+256). Band halo (max (KC-1)*dil = 248) is
loaded per-core with zero padding (projection of zero rows reproduces the
reference's bias padding exactly).

v2: fp16 end-to-end on the PE paths (inputs, weights, score plane, W plane),
xbar DMA-transpose readback of the W plane (replaces PE transposes + PSUM
copies), tight dil=8 W-plane (512 cols via paired 64-wide residue blocks).

Per-core pipeline (layouts noted as [partition, free]):
  1. Q/K projections (fp16 in, fp32 PSUM): qT_s/kT_s [dh=128, pos].
  2. Banded scores per subhead (fp32 PSUM): scores_s [q, m].
  3. Scores -> DRAM plane fp16 (deinterleaved by m%dil), shear-gather back as
     band [q, 32] (diagonal strided DMA, contiguous 32-element runs; rows
     permuted rho-major for dil>1), PE-transpose to bandT [32, q].
  4. Pos_Sampling (Sk fp16) matmul + Sb -> score2 [q, (head, 32)] fp32, exp,
     per-head row-sum, reciprocal, normalize -> W [q, (head, 32)] fp16.
  5. W scatter -> zeroed DRAM plane per head in [q, m] layout (contiguous runs,
     un-permuting the rho-major rows), xbar DMA-transpose readback ->
     W^T [m, q] fp16 tiles directly.
  6. V projection (fp16, heads packed) -> v_h [m, dh] tiles in plane-col order.
  7. PV: attnT_h [dh, q] = v_h^T @ W^T (fp32 accum) + Vb.
  8. Collapse: out [q, 640] = sum_h attnT_h^T @ CkT_h + Cb (fp16 weights).
"""

import contextlib
import ctypes
import sys
import types

import numpy as np

# ---------------------------------------------------------------- constants
B, N, D = 2, 1024, 640
DH, KC, SUBHEADS, HEADS = 128, 32, 5, 14
Q = 256                      # query positions per core
NCORES = 8
HALO = 124                   # (KC-1)*max_dil // 2
KV = 512                     # kv halo positions per core ([t0-124, t0+388))
KVP = 640                    # padded vT width (cols [512,640) memset to 0)

DIL_S = [1, 1, 2, 4, 8]
SUPER = [5, 5, 2, 1, 1]
DIL_H = [1] * 10 + [2, 2, 4, 8]
S_OF_H = [0] * 5 + [1] * 5 + [2, 2, 3, 4]
PL_S = [(KC - 1) * d // 2 for d in DIL_S]          # [15,15,31,62,124]
OFF_S = [HALO - p for p in PL_S]                   # kT col of m=0 per subhead
PL_H = [(KC - 1) * d // 2 for d in DIL_H]
OFF_H = [HALO - p for p in PL_H]

M_S = [288, 288, 320, 384, 512]                    # scores plane width per subhead
SOFF = [0, 288, 576, 896, 1280]
SLD = 1792                                         # scores plane row stride

# W plane width per head (fp16). Residue width per dilation:
#   d=1: single residue 320 (band needs cols [0,287)); d=2: 256/residue;
#   d=4: 128; d=8: 64 (paired per 128-chunk). Col c = rho*w + u holds
#   m = off + rho + d*u.
M_H = [320] * 10 + [512, 512, 512, 512]
W_RES = {1: 320, 2: 256, 4: 128, 8: 64}
WOFF = np.cumsum([0] + M_H).tolist()[:-1]
WLD = int(np.sum(M_H))                             # 5248
# (col_base, width) chunks per head for readback/PV
CHUNKS_H = [[(0, 128), (128, 128), (256, 64)]] * 10 + \
           [[(0, 128), (128, 128), (256, 128), (384, 128)]] * 4

# W plane split into independent DRAM tensors (per head group) so the
# scatters/readbacks of different groups carry no false WAW/RAW deps:
#   part 0: heads 0-4 (1600 cols), part 1: heads 5-9 (1600),
#   parts 2-5: heads 10-13 (512 each)
WPARTS = [(0, list(range(0, 5)), 1600), (1, list(range(5, 10)), 1600),
          (2, [10], 512), (3, [11], 512), (4, [12], 512), (5, [13], 512)]
PART_OF_H = {h: pi for pi, hs, _ in WPARTS for h in hs}
PWIDTH = {pi: w for pi, _, w in WPARTS}
# column base of head h inside its part tensor
PCOL_H = {h: (h - hs[0]) * 320 for pi, hs, _ in WPARTS for h in hs}

# V-projection head packs (same dilation within a pack)
PACKS = [[0, 1, 2, 3], [4, 5, 6, 7], [8, 9], [10, 11], [12], [13]]
PACK_OF_H = {h: (p, g.index(h)) for p, g in enumerate(PACKS) for h in g}
PACK_OFF = [OFF_H[g[0]] for g in PACKS]
PACK_DIL = [DIL_H[g[0]] for g in PACKS]
PACK_CHUNKS = [CHUNKS_H[g[0]] for g in PACKS]

# head-column layout in the 448-wide score2/W tiles
HJ = HEADS * KC  # 448

_BUILT = None


def _inject_ntff_hook():
    """bass_utils reads antenv.axon_hooks for NTFF profiling; the module is
    absent in this image. Recreate the ctypes glue (mirrors trn_boot.py)."""
    try:
        import antenv.axon_hooks  # noqa: F401
        return
    except ImportError:
        pass

    def _make(so_path):
        try:
            lib = ctypes.CDLL(so_path)
        except OSError:
            return None
        if not hasattr(lib, "axon_start_nrt_profile"):
            return None
        lib.axon_start_nrt_profile.argtypes = [ctypes.POINTER(ctypes.c_int64), ctypes.c_size_t]
        lib.axon_start_nrt_profile.restype = ctypes.c_int64
        lib.axon_stop_nrt_profile.argtypes = [ctypes.c_char_p]
        lib.axon_stop_nrt_profile.restype = ctypes.c_int64

        @contextlib.contextmanager
        def _hook(output_dir, device_ids):
            import jax
            jax.devices()
            if device_ids:
                ids = (ctypes.c_int64 * len(device_ids))(*device_ids)
                rc = lib.axon_start_nrt_profile(ids, len(device_ids))
            else:
                rc = lib.axon_start_nrt_profile(None, 0)
            if rc != 0:
                raise RuntimeError(f"axon_start_nrt_profile rc={rc}")
            try:
                yield
            finally:
                n = lib.axon_stop_nrt_profile(str(output_dir).encode())
                print(f"ntff profile: {n} file(s) -> {output_dir}", file=sys.stderr)

        return _hook

    hook = _make("/opt/axon/libaxon_pjrt.so")
    mod = types.ModuleType("antenv.axon_hooks")
    mod.get_axon_ntff_profile_hook = lambda: hook
    mod.set_axon_ntff_profile_hook = lambda h: None
    sys.modules["antenv.axon_hooks"] = mod


def _vchunk_ap(AP, vT, vs8, dc, p, cb, cw):
    """lhsT AP for V-projection: partitions = d_model chunk dc rows, free =
    the cw m-positions of plane-col chunk [cb, cb+cw) of pack p (plane-col
    order). Stationary APs allow only one free dim, so the dil=8 pack reads
    from the pre-staged vs8 tile (already in plane-col order)."""
    d, o = PACK_DIL[p], PACK_OFF[p]
    base = vT.offset + dc * KVP
    if d == 1:
        return AP(vT.tensor, base + o + cb, [[SUBHEADS * KVP, DH], [1, cw]])
    if d == 2:
        rho, u = divmod(cb, 256)
        return AP(vT.tensor, base + o + rho + 2 * u,
                  [[SUBHEADS * KVP, DH], [2, cw]])
    if d == 4:
        rho, u = divmod(cb, 128)
        return AP(vT.tensor, base + o + rho + 4 * u,
                  [[SUBHEADS * KVP, DH], [4, cw]])
    # d == 8: staged copy, cols already in plane order
    return AP(vs8.tensor, vs8.offset + dc * 512 + cb,
              [[SUBHEADS * 512, DH], [1, cw]])


def _build(debug=False):
    """Build the (single) SPMD Bass program. Returns finalized nc."""
    import concourse.bass as bass
    import concourse.tile as tile
    from concourse import bacc, mybir
    from concourse.masks import make_identity
    from concourse.tile import add_dep_helper

    f32 = mybir.dt.float32
    f16 = mybir.dt.float16
    AP = bass.AP

    nc = bacc.Bacc("TRN2", target_bir_lowering=False, debug=False, num_devices=NCORES)

    # ---------------- external IO
    qT_d = nc.dram_tensor("qT", [D, Q], f16, kind="ExternalInput")
    kT_d = nc.dram_tensor("kT", [D, KV], f16, kind="ExternalInput")
    vT_d = nc.dram_tensor("vT", [D, KV], f16, kind="ExternalInput")
    QkT_d = nc.dram_tensor("QkT", [SUBHEADS, D, DH], f16, kind="ExternalInput")
    KkT_d = nc.dram_tensor("KkT", [SUBHEADS, D, DH], f16, kind="ExternalInput")
    Vp_d = [nc.dram_tensor(f"VkT{p}", [D, len(g) * DH], f16, kind="ExternalInput")
            for p, g in enumerate(PACKS)]
    # rows 0..31 = Sk, row 32 = Sb (bias folded into the matmul via a ones-row)
    SkT_d = nc.dram_tensor("SkT", [KC + 1, HJ], f16, kind="ExternalInput")
    QbT_d = nc.dram_tensor("QbT", [DH, SUBHEADS], f32, kind="ExternalInput")
    KbT_d = nc.dram_tensor("KbT", [DH, SUBHEADS], f32, kind="ExternalInput")
    VbT_d = nc.dram_tensor("VbT", [DH, HEADS], f32, kind="ExternalInput")
    CkT_d = nc.dram_tensor("CkT", [HEADS * DH, D], f16, kind="ExternalInput")
    Cb_d = nc.dram_tensor("Cb", [1, D], f32, kind="ExternalInput")
    out_d = nc.dram_tensor("out", [Q, D], f32, kind="ExternalOutput")
    if debug:
        dbg_w = nc.dram_tensor("dbg_w", [Q, HJ], f32, kind="ExternalOutput")
        dbg_at = nc.dram_tensor("dbg_at", [HEADS * DH, Q], f32, kind="ExternalOutput")
        dbg_qk = nc.dram_tensor("dbg_qk", [SUBHEADS * DH, Q + KV], f32, kind="ExternalOutput")
        dbg_vt = nc.dram_tensor("dbg_vt", [WLD, DH], f32, kind="ExternalOutput")

    # ---------------- internal DRAM scratch (concrete offsets for shear APs)
    # per-subhead score planes: independent tensors -> no false WAW/RAW deps
    splanes = [nc.dram_tensor(f"splane{s}", [Q, M_S[s]], f16, kind="Internal")
               for s in range(SUBHEADS)]
    wparts = [nc.dram_tensor(f"wpart{pi}", [Q, w], f16, kind="Internal")
              for pi, _, w in WPARTS]

    with tile.TileContext(nc) as tc, contextlib.ExitStack() as ctx:
        consts = ctx.enter_context(tc.tile_pool(name="consts", bufs=1))
        acts = ctx.enter_context(tc.tile_pool(name="acts", bufs=1))
        work = ctx.enter_context(tc.tile_pool(name="work", bufs=4))
        wftp = ctx.enter_context(tc.tile_pool(name="wft", bufs=8))
        actp = ctx.enter_context(tc.tile_pool(name="actp", bufs=2))
        ps_mm = ctx.enter_context(tc.tile_pool(name="ps_mm", bufs=2, space="PSUM"))
        ps_sm = ctx.enter_context(tc.tile_pool(name="ps_sm", bufs=2, space="PSUM"))
        ps_at = ctx.enter_context(tc.tile_pool(name="ps_at", bufs=2, space="PSUM"))
        ps_co = ctx.enter_context(tc.tile_pool(name="ps_co", bufs=2, space="PSUM"))

        # ---------------- critical input loads (HWDGE queues)
        qTr = acts.tile([DH, SUBHEADS, Q], f16)
        kTr = acts.tile([DH, SUBHEADS, KV], f16)
        # per-subhead weight tiles so projections start as slices land
        QkTr = [consts.tile([DH, SUBHEADS, DH], f16, name=f"QkTr{s}") for s in range(SUBHEADS)]
        KkTr = [consts.tile([DH, SUBHEADS, DH], f16, name=f"KkTr{s}") for s in range(SUBHEADS)]
        nc.sync.dma_start(out=qTr, in_=AP(qT_d, 0, [[Q, DH], [DH * Q, SUBHEADS], [1, Q]]))
        nc.scalar.dma_start(out=kTr, in_=AP(kT_d, 0, [[KV, DH], [DH * KV, SUBHEADS], [1, KV]]))
        _gq = _gk = None
        for s in range(SUBHEADS):
            _gq = nc.sync.dma_start(
                out=QkTr[s],
                in_=AP(QkT_d, s * D * DH, [[DH, DH], [DH * DH, SUBHEADS], [1, DH]]))
            _gk = nc.scalar.dma_start(
                out=KkTr[s],
                in_=AP(KkT_d, s * D * DH, [[DH, DH], [DH * DH, SUBHEADS], [1, DH]]))
        QbT = consts.tile([DH, SUBHEADS], f32)
        nc.sync.dma_start(out=QbT, in_=QbT_d.ap())
        KbT = consts.tile([DH, SUBHEADS], f32)
        nc.scalar.dma_start(out=KbT, in_=KbT_d.ap())

        # ---------------- bulk loads (gpsimd, deferred behind ALL critical DMAs)
        def defer(dma):
            add_dep_helper(dma.ins, _gq.ins, sync=True,
                           reason="defer bulk DMA until critical q-path loaded")
            add_dep_helper(dma.ins, _gk.ins, sync=True,
                           reason="defer bulk DMA until critical k-path loaded")
            return dma

        # zero the W plane parts (needed by the first scatter, ~10us later)
        zr = work.tile([DH, 1600], f16, name="zr", tag="zr", bufs=1)
        nc.vector.memset(zr, 0.0)
        for pi, _, wd in WPARTS:
            for c in range(2):
                defer(nc.gpsimd.dma_start(
                    out=AP(wparts[pi], c * 128 * wd, [[wd, 128], [1, wd]]),
                    in_=zr[:, :wd],
                ))

        vT = acts.tile([DH, SUBHEADS, KVP], f16)
        nc.vector.memset(vT[:, :, KV:], 0.0)
        defer(nc.gpsimd.dma_start(
            out=AP(vT.tensor, vT.offset, [[SUBHEADS * KVP, DH], [KVP, SUBHEADS], [1, KV]]),
            in_=AP(vT_d, 0, [[KV, DH], [DH * KV, SUBHEADS], [1, KV]])))
        Vp = []
        for p, g in enumerate(PACKS):
            npk = len(g) * DH
            t = consts.tile([DH, SUBHEADS, npk], f16, name=f"Vp{p}")
            defer(nc.gpsimd.dma_start(
                out=t, in_=AP(Vp_d[p], 0, [[npk, DH], [DH * npk, SUBHEADS], [1, npk]])))
            Vp.append(t)
        SkT = consts.tile([KC + 1, HJ], f16)
        defer(nc.gpsimd.dma_start(out=SkT, in_=SkT_d.ap()))
        VbT = consts.tile([DH, HEADS], f32)
        defer(nc.gpsimd.dma_start(out=VbT, in_=VbT_d.ap()))

        ident = consts.tile([DH, DH], f32)
        make_identity(nc, ident)
        identh = consts.tile([DH, DH], f16)
        nc.vector.tensor_copy(identh, ident)

        # ---------------- Q/K projections (fp16 in, fp32 PSUM, fp16 out)
        qTs, kTs = [], []
        for s in range(SUBHEADS):
            pq = ps_mm.tile([DH, Q], f32, name=f"pq{s}", tag="mm")
            for dc in range(SUBHEADS):
                nc.tensor.matmul(pq, QkTr[s][:, dc, :], qTr[:, dc, :],
                                 start=(dc == 0), stop=(dc == SUBHEADS - 1))
            t = acts.tile([DH, Q], f16, name=f"qTs{s}")
            nc.scalar.activation(t, pq, mybir.ActivationFunctionType.Identity,
                                 bias=QbT[:, s : s + 1], scale=1.0)
            qTs.append(t)

            pk = ps_mm.tile([DH, KV], f32, name=f"pk{s}", tag="mm")
            for dc in range(SUBHEADS):
                nc.tensor.matmul(pk, KkTr[s][:, dc, :], kTr[:, dc, :],
                                 start=(dc == 0), stop=(dc == SUBHEADS - 1))
            t = acts.tile([DH, KV], f16, name=f"kTs{s}")
            nc.scalar.activation(t, pk, mybir.ActivationFunctionType.Identity,
                                 bias=KbT[:, s : s + 1], scale=1.0)
            kTs.append(t)

        if debug:
            for s in range(SUBHEADS):
                tq = work.tile([DH, Q], f32, name=f"dq{s}", tag="dbg", bufs=2)
                nc.vector.tensor_copy(tq, qTs[s])
                nc.sync.dma_start(
                    out=AP(dbg_qk, s * DH * (Q + KV), [[Q + KV, DH], [1, Q]]), in_=tq)
                tk = work.tile([DH, KV], f32, name=f"dk{s}", tag="dbg", bufs=2)
                nc.vector.tensor_copy(tk, kTs[s])
                nc.sync.dma_start(
                    out=AP(dbg_qk, s * DH * (Q + KV) + Q, [[Q + KV, DH], [1, KV]]), in_=tk)

        # ---------------- banded scores -> deinterleaved DRAM plane (fp16)
        sdma = [nc.sync, nc.scalar]
        for c in range(2):
            ssb = work.tile([128, SLD], f16, name="ssb", tag="ssb", bufs=2)
            for s in range(SUBHEADS):
                dil, ms = DIL_S[s], M_S[s]
                pscore = ps_mm.tile([128, ms], f32, name=f"psc{s}{c}", tag="mm")
                nc.tensor.matmul(pscore, qTs[s][:, c * 128 : c * 128 + 128],
                                 kTs[s][:, OFF_S[s] : OFF_S[s] + ms],
                                 start=True, stop=True)
                if dil == 1:
                    if c == 0:
                        nc.scalar.copy(ssb[:, SOFF[s] : SOFF[s] + ms], pscore)
                    else:
                        nc.vector.tensor_copy(ssb[:, SOFF[s] : SOFF[s] + ms], pscore)
                else:
                    # deinterleave m -> (m%dil, m//dil) during PSUM->SBUF copy
                    psrc = AP(pscore.tensor, pscore.offset,
                              [[ms, 128], [1, dil], [dil, ms // dil]])
                    dst = AP(ssb.tensor, ssb.offset + SOFF[s],
                             [[SLD, 128], [ms // dil, dil], [1, ms // dil]])
                    if c == 0:
                        nc.scalar.copy(dst, psrc)
                    else:
                        nc.vector.tensor_copy(dst, psrc)
                # per-subhead plane write for earlier downstream starts
                sdma[c].dma_start(
                    out=AP(splanes[s], c * 128 * ms, [[ms, 128], [1, ms]]),
                    in_=ssb[:, SOFF[s] : SOFF[s] + ms])

        # ---------------- V projection emit helper (interleaved into the
        # softmax gather-latency windows to keep the PE queue fed)
        # stage dil=8 vT columns into plane-col order (stationary APs are 2-dim only)
        vs8 = acts.tile([DH, SUBHEADS, 512], f16, name="vs8")
        o8 = PACK_OFF[5]
        for dc in range(SUBHEADS):
            nc.gpsimd.tensor_copy(
                AP(vs8.tensor, vs8.offset + dc * 512,
                   [[SUBHEADS * 512, DH], [128, 4], [64, 2], [1, 64]]),
                AP(vT.tensor, vT.offset + dc * KVP + o8,
                   [[SUBHEADS * KVP, DH], [2, 4], [1, 2], [8, 64]]),
            )

        vtiles = {}  # (pack, mc) -> [cw, len(g)*128] f16

        def emit_vproj(packs):
            for p in packs:
                g = PACKS[p]
                npk = len(g) * DH
                for mc, (cb, cw) in enumerate(PACK_CHUNKS[p]):
                    pv = ps_mm.tile([cw, npk], f32, name=f"pv{p}{mc}", tag="mm")
                    for dc in range(SUBHEADS):
                        nc.tensor.matmul(pv, _vchunk_ap(AP, vT, vs8, dc, p, cb, cw),
                                         Vp[p][:, dc, :],
                                         start=(dc == 0), stop=(dc == SUBHEADS - 1))
                    t = acts.tile([cw, npk], f16, name=f"v{p}_{mc}")
                    if (p + mc) % 2 == 0:
                        nc.vector.tensor_copy(t, pv)
                    else:
                        nc.scalar.copy(t, pv)
                    vtiles[(p, mc)] = t
                    if debug:
                        for gi, h in enumerate(g):
                            dv = work.tile([cw, DH], f32, name="dv", tag="dbg", bufs=2)
                            nc.vector.tensor_copy(dv, t[:, gi * DH : gi * DH + DH])
                            nc.sync.dma_start(
                                out=AP(dbg_vt, (WOFF[h] + cb) * DH, [[DH, cw], [1, DH]]),
                                in_=dv)

        emit_vproj([3, 4, 5])

        # ---------------- band extract + Sk + softmax -> W (fp16)
        for c in range(2):
            if c == 1:
                emit_vproj([0, 1, 2])
            bandTs = []
            for s in range(SUBHEADS):
                dil, ms = DIL_S[s], M_S[s]
                band = work.tile([128, KC], f16, name="band", tag="band", bufs=6)
                if dil == 1:
                    _g = sdma[c].dma_start(
                        out=band,
                        in_=AP(splanes[s], c * 128 * (ms + 1),
                               [[ms + 1, 128], [1, KC]]))
                    if c == 0 and s == 0:
                        _gath0 = _g
                else:
                    # one gather; band rows permuted rho-major:
                    # row rho*(128/dil)+P holds query q = dil*P + rho
                    sdma[c].dma_start(
                        out=band,
                        in_=AP(splanes[s],
                               c * 128 * ms + (c * 128) // dil,
                               [[ms + ms // dil, dil], [dil * ms + 1, 128 // dil], [1, KC]]))
                pbt = ps_sm.tile([KC, 128], f16, name="pbt", tag="sm")
                nc.tensor.transpose(pbt, band, identh)
                # row 32 = ones (pairs with the Sb row folded into SkT's last row)
                bt = work.tile([KC + 1, 128], f16, name="bt", tag="bt", bufs=5)
                nc.vector.memset(bt[KC : KC + 1], 1.0)
                if c == 0:
                    nc.scalar.copy(bt[:KC], pbt)
                else:
                    nc.vector.tensor_copy(bt[:KC], pbt)
                bandTs.append(bt)

            e = work.tile([128, HJ], f32, name="e", tag="e", bufs=2)
            psk = ps_mm.tile([128, HJ], f32, name="psk", tag="mm")
            hlo = 0
            for s in range(SUBHEADS):
                ncols = SUPER[s] * KC
                nc.tensor.matmul(psk[:, hlo : hlo + ncols], bandTs[s],
                                 SkT[:, hlo : hlo + ncols],
                                 start=True, stop=True, skip_group_check=True)
                hlo += ncols
            nc.scalar.activation(e, psk, mybir.ActivationFunctionType.Exp)
            z = work.tile([128, HEADS], f32, name="z", tag="z", bufs=4)
            nc.vector.reduce_sum(z, e.rearrange("p (h k) -> p h k", k=KC),
                                 axis=mybir.AxisListType.X)
            rz = work.tile([128, HEADS], f32, name="rz", tag="z", bufs=4)
            nc.vector.reciprocal(rz, z)
            w = work.tile([128, HJ], f16, name="w", tag="w", bufs=2)
            nc.vector.tensor_mul(
                w.rearrange("p (h k) -> p h k", k=KC),
                e.rearrange("p (h k) -> p h k", k=KC),
                AP(rz.tensor, rz.offset, [[HEADS, 128], [1, HEADS], [0, KC]]),
            )

            if debug:
                dw = work.tile([128, HJ], f32, name="dw", tag="dbg", bufs=2)
                nc.vector.tensor_copy(dw, w)
                nc.sync.dma_start(
                    out=AP(dbg_w, c * 128 * HJ, [[HJ, 128], [1, HJ]]), in_=dw)

            # ---- scatter W into the zeroed plane parts ([q, m] layout,
            # contiguous runs). Each part is its own DRAM tensor, so the six
            # scatters per chunk are fully independent.
            for half in range(2):
                wd = 1600
                sdma[half].dma_start(
                    out=AP(wparts[half], c * 128 * (wd + 1),
                           [[wd + 1, 128], [320, 5], [1, KC]]),
                    in_=AP(w.tensor, w.offset + half * 5 * KC,
                           [[HJ, 128], [KC, 5], [1, KC]]),
                )
            for h in range(10, HEADS):
                dil = DIL_H[h]
                wres = W_RES[dil]
                wd = 512
                base = c * 128 * wd + (c * 128) // dil
                sdma[h % 2].dma_start(
                    out=AP(wparts[PART_OF_H[h]], base,
                           [[wd + wres, dil], [dil * wd + 1, 128 // dil], [1, KC]]),
                    in_=AP(w.tensor, w.offset + h * KC, [[HJ, 128], [1, KC]]),
                )

        # ---------------- W plane bulk readback (per part, pipelined) + PV
        pcol = {0: 0, 1: 1600, 2: 3200, 3: 3712, 4: 4224, 5: 4736}
        wpl = []
        for c in range(2):
            t = acts.tile([128, WLD], f16, name=f"wpl{c}")
            # single-head parts first: their scatters drain fastest, letting
            # PV on heads 10-13 start while the 5-head scatters finish
            for pi, hs, wd in [WPARTS[i] for i in (2, 3, 4, 5, 0, 1)]:
                sdma[pi % 2].dma_start(
                    out=t[:, pcol[pi] : pcol[pi] + wd],
                    in_=AP(wparts[pi], c * 128 * wd, [[wd, 128], [1, wd]]))
            wpl.append(t)

        # collapse weights: load during the softmax window (DMA slack there)
        CkT = consts.tile([DH, HEADS, D], f16)   # f-chunk h on partitions' free dim
        _ck = nc.gpsimd.dma_start(
            out=CkT, in_=AP(CkT_d, 0, [[D, DH], [DH * D, HEADS], [1, D]]))
        add_dep_helper(_ck.ins, _gath0.ins, sync=True,
                       reason="defer collapse weights into the softmax DMA window")
        Cb = consts.tile([DH, D], f32)
        _cb = nc.gpsimd.dma_start(out=Cb, in_=AP(Cb_d, 0, [[0, DH], [1, D]]))
        add_dep_helper(_cb.ins, _gath0.ins, sync=True,
                       reason="defer collapse bias into the softmax DMA window")
        cpeng = [nc.vector, nc.vector]
        atiles = [None] * HEADS
        for h in [10, 11, 12, 13] + list(range(10)):
            p, hh = PACK_OF_H[h]
            chunks = CHUNKS_H[h]
            pat = ps_at.tile([DH, Q], f32, name=f"pat{h}", tag="at")
            for mc, (cb, cw) in enumerate(chunks):
                ptp = ps_sm.tile([128, Q], f16, name="ptp", tag="sm")
                for c in range(2):
                    nc.tensor.transpose(
                        ptp[:cw, c * 128 : c * 128 + 128],
                        wpl[c][:, WOFF[h] + cb : WOFF[h] + cb + cw],
                        identh)
                wft = wftp.tile([128, Q], f16, name="wft", tag="wft")
                cpeng[(h + mc) % 2].tensor_copy(wft[:cw], ptp[:cw])
                nc.tensor.matmul(pat, vtiles[(p, mc)][:, hh * DH : hh * DH + DH],
                                 wft[:cw],
                                 start=(mc == 0), stop=(mc == len(chunks) - 1))
            at = actp.tile([DH, Q], f16, name=f"at{h}", tag="at", bufs=14)
            nc.scalar.activation(at, pat, mybir.ActivationFunctionType.Identity,
                                 bias=VbT[:, h : h + 1], scale=1.0)
            atiles[h] = at
            if debug:
                da = work.tile([DH, Q], f32, name="da", tag="dbg", bufs=2)
                nc.vector.tensor_copy(da, at)
                nc.sync.dma_start(
                    out=AP(dbg_at, h * DH * Q, [[Q, DH], [1, Q]]), in_=da)

        # ---------------- collapse + output
        outsb = [work.tile([128, D], f32, name=f"osb{c}", tag="osb", bufs=2) for c in range(2)]
        for c in range(2):
            for half in range(2):
                pc = ps_co.tile([128, 320], f32, name=f"pc{c}{half}", tag="co")
                for h in range(HEADS):
                    nc.tensor.matmul(pc, atiles[h][:, c * 128 : c * 128 + 128],
                                     CkT[:, h, half * 320 : half * 320 + 320],
                                     start=(h == 0), stop=(h == HEADS - 1))
                nc.vector.tensor_add(
                    outsb[c][:, half * 320 : half * 320 + 320], pc,
                    Cb[:, half * 320 : half * 320 + 320],
                )
            nc.sync.dma_start(
                out=AP(out_d, c * 128 * D, [[D, 128], [1, D]]),
                in_=outsb[c],
            )

    nc.finalize()
    return nc


def _prep_in_maps(inputs):
    f16 = np.float16
    query = np.asarray(inputs["query"], np.float32)
    key = np.asarray(inputs["key"], np.float32)
    value = np.asarray(inputs["value"], np.float32)
    Qk = np.asarray(inputs["Qk"], np.float32)
    Qb = np.asarray(inputs["Qb"], np.float32)
    Kk = np.asarray(inputs["Kk"], np.float32)
    Kb = np.asarray(inputs["Kb"], np.float32)
    Vk = np.asarray(inputs["Vk"], np.float32)
    Vb = np.asarray(inputs["Vb"], np.float32)
    Sk = np.asarray(inputs["Sk"], np.float32)
    Sb = np.asarray(inputs["Sb"], np.float32)
    Ck = np.asarray(inputs["Ck"], np.float32)
    Cb = np.asarray(inputs["Cb"], np.float32)

    QkT = np.ascontiguousarray(Qk.transpose(0, 2, 1)).astype(f16)  # [5, 640, 128]
    KkT = np.ascontiguousarray(Kk.transpose(0, 2, 1)).astype(f16)
    VkT = Vk.transpose(0, 2, 1)                                    # [14, 640, 128]
    Vp = [np.ascontiguousarray(
            np.concatenate([VkT[h] for h in g], axis=1)).astype(f16)
          for g in PACKS]
    SkT = np.concatenate(
        [Sk.transpose(2, 0, 1).reshape(KC, HJ), Sb.reshape(1, HJ)], axis=0
    ).astype(f16)                                                  # [33, 448]
    QbT = np.ascontiguousarray(Qb.T)                               # [128, 5]
    KbT = np.ascontiguousarray(Kb.T)
    VbT = np.ascontiguousarray(Vb.T)                               # [128, 14]
    CkT = np.ascontiguousarray(Ck.T).astype(f16)                   # [1792, 640]
    Cbr = np.ascontiguousarray(Cb.reshape(1, D))

    in_maps = []
    for c in range(NCORES):
        b, t0 = c // 4, (c % 4) * Q
        kpad = np.zeros((KV, D), np.float32)
        vpad = np.zeros((KV, D), np.float32)
        lo, hi = max(0, t0 - HALO), min(N, t0 + Q + 132)
        kpad[lo - (t0 - HALO) : hi - (t0 - HALO)] = key[b, lo:hi]
        vpad[lo - (t0 - HALO) : hi - (t0 - HALO)] = value[b, lo:hi]
        m = {
            "qT": np.ascontiguousarray(query[b, t0 : t0 + Q].T).astype(f16),
            "kT": np.ascontiguousarray(kpad.T).astype(f16),
            "vT": np.ascontiguousarray(vpad.T).astype(f16),
            "QkT": QkT, "KkT": KkT,
            "SkT": SkT, "QbT": QbT, "KbT": KbT, "VbT": VbT,
            "CkT": CkT, "Cb": Cbr,
        }
        for p in range(len(PACKS)):
            m[f"VkT{p}"] = Vp[p]
        in_maps.append(m)
    return in_maps


def _run(inputs, trace=False, tmpdir=None):
    global _BUILT
    _inject_ntff_hook()
    from concourse.bass_utils import run_bass_kernel_spmd

    if _BUILT is None:
        _BUILT = _build()
    in_maps = _prep_in_maps(inputs)
    r = run_bass_kernel_spmd(_BUILT, in_maps, core_ids=list(range(NCORES)),
                             trace=trace, tmpdir=tmpdir)
    out = np.empty((B, N, D), np.float32)
    for c in range(NCORES):
        b, t0 = c // 4, (c % 4) * Q
        out[b, t0 : t0 + Q] = r.results[c]["out"]
    return out, r


def kernel(**inputs) -> np.ndarray:
    out, _ = _run(inputs, trace=False)
    return out


# revision 77
# speedup vs baseline: 1.0448x; 1.0448x over previous
"""Banded multi-headed attention (nn_BandedMultiheadedAttention) on 8 Trainium2 NeuronCores.

Sharding: data-parallel over (batch, sequence-chunk): core c handles batch c//4,
query positions [256*(c%4), 256*(c%4)+256). Band halo (max (KC-1)*dil = 248) is
loaded per-core with zero padding (projection of zero rows reproduces the
reference's bias padding exactly).

v2: fp16 end-to-end on the PE paths (inputs, weights, score plane, W plane),
xbar DMA-transpose readback of the W plane (replaces PE transposes + PSUM
copies), tight dil=8 W-plane (512 cols via paired 64-wide residue blocks).

Per-core pipeline (layouts noted as [partition, free]):
  1. Q/K projections (fp16 in, fp32 PSUM): qT_s/kT_s [dh=128, pos].
  2. Banded scores per subhead (fp32 PSUM): scores_s [q, m].
  3. Scores -> DRAM plane fp16 (deinterleaved by m%dil), shear-gather back as
     band [q, 32] (diagonal strided DMA, contiguous 32-element runs; rows
     permuted rho-major for dil>1), PE-transpose to bandT [32, q].
  4. Pos_Sampling (Sk fp16) matmul + Sb -> score2 [q, (head, 32)] fp32, exp,
     per-head row-sum, reciprocal, normalize -> W [q, (head, 32)] fp16.
  5. W scatter -> zeroed DRAM plane per head in [q, m] layout (contiguous runs,
     un-permuting the rho-major rows), xbar DMA-transpose readback ->
     W^T [m, q] fp16 tiles directly.
  6. V projection (fp16, heads packed) -> v_h [m, dh] tiles in plane-col order.
  7. PV: attnT_h [dh, q] = v_h^T @ W^T (fp32 accum) + Vb.
  8. Collapse: out [q, 640] = sum_h attnT_h^T @ CkT_h + Cb (fp16 weights).
"""

import contextlib
import ctypes
import sys
import types

import numpy as np

# ---------------------------------------------------------------- constants
B, N, D = 2, 1024, 640
DH, KC, SUBHEADS, HEADS = 128, 32, 5, 14
Q = 256                      # query positions per core
NCORES = 8
HALO = 124                   # (KC-1)*max_dil // 2
KV = 512                     # kv halo positions per core ([t0-124, t0+388))
KVP = 640                    # padded vT width (cols [512,640) memset to 0)

DIL_S = [1, 1, 2, 4, 8]
SUPER = [5, 5, 2, 1, 1]
DIL_H = [1] * 10 + [2, 2, 4, 8]
S_OF_H = [0] * 5 + [1] * 5 + [2, 2, 3, 4]
PL_S = [(KC - 1) * d // 2 for d in DIL_S]          # [15,15,31,62,124]
OFF_S = [HALO - p for p in PL_S]                   # kT col of m=0 per subhead
PL_H = [(KC - 1) * d // 2 for d in DIL_H]
OFF_H = [HALO - p for p in PL_H]

M_S = [288, 288, 320, 384, 512]                    # scores plane width per subhead
SOFF = [0, 288, 576, 896, 1280]
SLD = 1792                                         # scores plane row stride

# W plane width per head (fp16). Residue width per dilation:
#   d=1: single residue 320 (band needs cols [0,287)); d=2: 256/residue;
#   d=4: 128; d=8: 64 (paired per 128-chunk). Col c = rho*w + u holds
#   m = off + rho + d*u.
M_H = [320] * 10 + [512, 512, 512, 512]
W_RES = {1: 320, 2: 256, 4: 128, 8: 64}
WOFF = np.cumsum([0] + M_H).tolist()[:-1]
WLD = int(np.sum(M_H))                             # 5248
# (col_base, width) chunks per head for readback/PV
CHUNKS_H = [[(0, 128), (128, 128), (256, 64)]] * 10 + \
           [[(0, 128), (128, 128), (256, 128), (384, 128)]] * 4

# W plane split into independent DRAM tensors (per head group) so the
# scatters/readbacks of different groups carry no false WAW/RAW deps:
#   part 0: heads 0-4 (1600 cols), part 1: heads 5-9 (1600),
#   parts 2-5: heads 10-13 (512 each)
WPARTS = [(0, list(range(0, 5)), 1600), (1, list(range(5, 10)), 1600),
          (2, [10], 512), (3, [11], 512), (4, [12], 512), (5, [13], 512)]
PART_OF_H = {h: pi for pi, hs, _ in WPARTS for h in hs}
PWIDTH = {pi: w for pi, _, w in WPARTS}
# column base of head h inside its part tensor
PCOL_H = {h: (h - hs[0]) * 320 for pi, hs, _ in WPARTS for h in hs}

# V-projection head packs (same dilation within a pack)
PACKS = [[0, 1, 2, 3], [4, 5, 6, 7], [8, 9], [10, 11], [12], [13]]
PACK_OF_H = {h: (p, g.index(h)) for p, g in enumerate(PACKS) for h in g}
PACK_OFF = [OFF_H[g[0]] for g in PACKS]
PACK_DIL = [DIL_H[g[0]] for g in PACKS]
PACK_CHUNKS = [CHUNKS_H[g[0]] for g in PACKS]

# head-column layout in the 448-wide score2/W tiles
HJ = HEADS * KC  # 448

_BUILT = None


def _inject_ntff_hook():
    """bass_utils reads antenv.axon_hooks for NTFF profiling; the module is
    absent in this image. Recreate the ctypes glue (mirrors trn_boot.py)."""
    try:
        import antenv.axon_hooks  # noqa: F401
        return
    except ImportError:
        pass

    def _make(so_path):
        try:
            lib = ctypes.CDLL(so_path)
        except OSError:
            return None
        if not hasattr(lib, "axon_start_nrt_profile"):
            return None
        lib.axon_start_nrt_profile.argtypes = [ctypes.POINTER(ctypes.c_int64), ctypes.c_size_t]
        lib.axon_start_nrt_profile.restype = ctypes.c_int64
        lib.axon_stop_nrt_profile.argtypes = [ctypes.c_char_p]
        lib.axon_stop_nrt_profile.restype = ctypes.c_int64

        @contextlib.contextmanager
        def _hook(output_dir, device_ids):
            import jax
            jax.devices()
            if device_ids:
                ids = (ctypes.c_int64 * len(device_ids))(*device_ids)
                rc = lib.axon_start_nrt_profile(ids, len(device_ids))
            else:
                rc = lib.axon_start_nrt_profile(None, 0)
            if rc != 0:
                raise RuntimeError(f"axon_start_nrt_profile rc={rc}")
            try:
                yield
            finally:
                n = lib.axon_stop_nrt_profile(str(output_dir).encode())
                print(f"ntff profile: {n} file(s) -> {output_dir}", file=sys.stderr)

        return _hook

    hook = _make("/opt/axon/libaxon_pjrt.so")
    mod = types.ModuleType("antenv.axon_hooks")
    mod.get_axon_ntff_profile_hook = lambda: hook
    mod.set_axon_ntff_profile_hook = lambda h: None
    sys.modules["antenv.axon_hooks"] = mod


def _vchunk_ap(AP, vT, vs8, dc, p, cb, cw):
    """lhsT AP for V-projection: partitions = d_model chunk dc rows, free =
    the cw m-positions of plane-col chunk [cb, cb+cw) of pack p (plane-col
    order). Stationary APs allow only one free dim, so the dil=8 pack reads
    from the pre-staged vs8 tile (already in plane-col order)."""
    d, o = PACK_DIL[p], PACK_OFF[p]
    base = vT.offset + dc * KVP
    if d == 1:
        return AP(vT.tensor, base + o + cb, [[SUBHEADS * KVP, DH], [1, cw]])
    if d == 2:
        rho, u = divmod(cb, 256)
        return AP(vT.tensor, base + o + rho + 2 * u,
                  [[SUBHEADS * KVP, DH], [2, cw]])
    if d == 4:
        rho, u = divmod(cb, 128)
        return AP(vT.tensor, base + o + rho + 4 * u,
                  [[SUBHEADS * KVP, DH], [4, cw]])
    # d == 8: staged copy, cols already in plane order
    return AP(vs8.tensor, vs8.offset + dc * 512 + cb,
              [[SUBHEADS * 512, DH], [1, cw]])


def _build(debug=False):
    """Build the (single) SPMD Bass program. Returns finalized nc."""
    import concourse.bass as bass
    import concourse.tile as tile
    from concourse import bacc, mybir
    from concourse.masks import make_identity
    from concourse.tile import add_dep_helper

    f32 = mybir.dt.float32
    f16 = mybir.dt.float16
    AP = bass.AP

    nc = bacc.Bacc("TRN2", target_bir_lowering=False, debug=False, num_devices=NCORES)

    # ---------------- external IO
    qT_d = nc.dram_tensor("qT", [D, Q], f16, kind="ExternalInput")
    kT_d = nc.dram_tensor("kT", [D, KV], f16, kind="ExternalInput")
    vT_d = nc.dram_tensor("vT", [D, KV], f16, kind="ExternalInput")
    QkT_d = nc.dram_tensor("QkT", [SUBHEADS, D, DH], f16, kind="ExternalInput")
    KkT_d = nc.dram_tensor("KkT", [SUBHEADS, D, DH], f16, kind="ExternalInput")
    Vp_d = [nc.dram_tensor(f"VkT{p}", [D, len(g) * DH], f16, kind="ExternalInput")
            for p, g in enumerate(PACKS)]
    # rows 0..31 = Sk, row 32 = Sb (bias folded into the matmul via a ones-row)
    SkT_d = nc.dram_tensor("SkT", [KC + 1, HJ], f16, kind="ExternalInput")
    QbT_d = nc.dram_tensor("QbT", [DH, SUBHEADS], f32, kind="ExternalInput")
    KbT_d = nc.dram_tensor("KbT", [DH, SUBHEADS], f32, kind="ExternalInput")
    CkT_d = nc.dram_tensor("CkT", [HEADS * DH, D], f16, kind="ExternalInput")
    Cb_d = nc.dram_tensor("Cb", [1, D], f32, kind="ExternalInput")
    out_d = nc.dram_tensor("out", [Q, D], f32, kind="ExternalOutput")
    if debug:
        dbg_w = nc.dram_tensor("dbg_w", [Q, HJ], f32, kind="ExternalOutput")
        dbg_at = nc.dram_tensor("dbg_at", [HEADS * DH, Q], f32, kind="ExternalOutput")
        dbg_qk = nc.dram_tensor("dbg_qk", [SUBHEADS * DH, Q + KV], f32, kind="ExternalOutput")
        dbg_vt = nc.dram_tensor("dbg_vt", [WLD, DH], f32, kind="ExternalOutput")

    # ---------------- internal DRAM scratch (concrete offsets for shear APs)
    # per-subhead score planes: independent tensors -> no false WAW/RAW deps
    splanes = [nc.dram_tensor(f"splane{s}", [Q, M_S[s]], f16, kind="Internal")
               for s in range(SUBHEADS)]
    wparts = [nc.dram_tensor(f"wpart{pi}", [Q, w], f16, kind="Internal")
              for pi, _, w in WPARTS]

    with tile.TileContext(nc) as tc, contextlib.ExitStack() as ctx:
        consts = ctx.enter_context(tc.tile_pool(name="consts", bufs=1))
        acts = ctx.enter_context(tc.tile_pool(name="acts", bufs=1))
        work = ctx.enter_context(tc.tile_pool(name="work", bufs=4))
        wftp = ctx.enter_context(tc.tile_pool(name="wft", bufs=8))
        actp = ctx.enter_context(tc.tile_pool(name="actp", bufs=2))
        ps_mm = ctx.enter_context(tc.tile_pool(name="ps_mm", bufs=2, space="PSUM"))
        ps_sm = ctx.enter_context(tc.tile_pool(name="ps_sm", bufs=2, space="PSUM"))
        ps_at = ctx.enter_context(tc.tile_pool(name="ps_at", bufs=2, space="PSUM"))
        ps_co = ctx.enter_context(tc.tile_pool(name="ps_co", bufs=2, space="PSUM"))

        # ---------------- critical input loads (HWDGE queues)
        qTr = acts.tile([DH, SUBHEADS, Q], f16)
        kTr = acts.tile([DH, SUBHEADS, KV], f16)
        # per-subhead weight tiles so projections start as slices land
        QkTr = [consts.tile([DH, SUBHEADS, DH], f16, name=f"QkTr{s}") for s in range(SUBHEADS)]
        KkTr = [consts.tile([DH, SUBHEADS, DH], f16, name=f"KkTr{s}") for s in range(SUBHEADS)]
        nc.sync.dma_start(out=qTr, in_=AP(qT_d, 0, [[Q, DH], [DH * Q, SUBHEADS], [1, Q]]))
        nc.scalar.dma_start(out=kTr, in_=AP(kT_d, 0, [[KV, DH], [DH * KV, SUBHEADS], [1, KV]]))
        _gq = _gk = None
        for s in range(SUBHEADS):
            _gq = nc.sync.dma_start(
                out=QkTr[s],
                in_=AP(QkT_d, s * D * DH, [[DH, DH], [DH * DH, SUBHEADS], [1, DH]]))
            _gk = nc.scalar.dma_start(
                out=KkTr[s],
                in_=AP(KkT_d, s * D * DH, [[DH, DH], [DH * DH, SUBHEADS], [1, DH]]))
        QbT = consts.tile([DH, SUBHEADS], f32)
        nc.sync.dma_start(out=QbT, in_=QbT_d.ap())
        KbT = consts.tile([DH, SUBHEADS], f32)
        nc.scalar.dma_start(out=KbT, in_=KbT_d.ap())

        # ---------------- bulk loads (gpsimd, deferred behind ALL critical DMAs)
        def defer(dma):
            add_dep_helper(dma.ins, _gq.ins, sync=True,
                           reason="defer bulk DMA until critical q-path loaded")
            add_dep_helper(dma.ins, _gk.ins, sync=True,
                           reason="defer bulk DMA until critical k-path loaded")
            return dma

        # zero the W plane parts (needed by the first scatter, ~10us later)
        zr = work.tile([DH, 1600], f16, name="zr", tag="zr", bufs=1)
        nc.vector.memset(zr, 0.0)
        for pi, _, wd in WPARTS:
            for c in range(2):
                defer(nc.gpsimd.dma_start(
                    out=AP(wparts[pi], c * 128 * wd, [[wd, 128], [1, wd]]),
                    in_=zr[:, :wd],
                ))

        vT = acts.tile([DH, SUBHEADS, KVP], f16)
        nc.vector.memset(vT[:, :, KV:], 0.0)
        defer(nc.gpsimd.dma_start(
            out=AP(vT.tensor, vT.offset, [[SUBHEADS * KVP, DH], [KVP, SUBHEADS], [1, KV]]),
            in_=AP(vT_d, 0, [[KV, DH], [DH * KV, SUBHEADS], [1, KV]])))
        Vp = []
        for p, g in enumerate(PACKS):
            npk = len(g) * DH
            t = consts.tile([DH, SUBHEADS, npk], f16, name=f"Vp{p}")
            defer(nc.gpsimd.dma_start(
                out=t, in_=AP(Vp_d[p], 0, [[npk, DH], [DH * npk, SUBHEADS], [1, npk]])))
            Vp.append(t)
        SkT = consts.tile([KC + 1, HJ], f16)
        defer(nc.gpsimd.dma_start(out=SkT, in_=SkT_d.ap()))


        ident = consts.tile([DH, DH], f32)
        make_identity(nc, ident)
        identh = consts.tile([DH, DH], f16)
        nc.vector.tensor_copy(identh, ident)

        # ---------------- Q/K projections (fp16 in, fp32 PSUM, fp16 out)
        qTs, kTs = [], []
        for s in range(SUBHEADS):
            pq = ps_mm.tile([DH, Q], f32, name=f"pq{s}", tag="mm")
            for dc in range(SUBHEADS):
                nc.tensor.matmul(pq, QkTr[s][:, dc, :], qTr[:, dc, :],
                                 start=(dc == 0), stop=(dc == SUBHEADS - 1))
            t = acts.tile([DH, Q], f16, name=f"qTs{s}")
            nc.scalar.activation(t, pq, mybir.ActivationFunctionType.Identity,
                                 bias=QbT[:, s : s + 1], scale=1.0)
            qTs.append(t)

            pk = ps_mm.tile([DH, KV], f32, name=f"pk{s}", tag="mm")
            for dc in range(SUBHEADS):
                nc.tensor.matmul(pk, KkTr[s][:, dc, :], kTr[:, dc, :],
                                 start=(dc == 0), stop=(dc == SUBHEADS - 1))
            t = acts.tile([DH, KV], f16, name=f"kTs{s}")
            nc.scalar.activation(t, pk, mybir.ActivationFunctionType.Identity,
                                 bias=KbT[:, s : s + 1], scale=1.0)
            kTs.append(t)

        if debug:
            for s in range(SUBHEADS):
                tq = work.tile([DH, Q], f32, name=f"dq{s}", tag="dbg", bufs=2)
                nc.vector.tensor_copy(tq, qTs[s])
                nc.sync.dma_start(
                    out=AP(dbg_qk, s * DH * (Q + KV), [[Q + KV, DH], [1, Q]]), in_=tq)
                tk = work.tile([DH, KV], f32, name=f"dk{s}", tag="dbg", bufs=2)
                nc.vector.tensor_copy(tk, kTs[s])
                nc.sync.dma_start(
                    out=AP(dbg_qk, s * DH * (Q + KV) + Q, [[Q + KV, DH], [1, KV]]), in_=tk)

        # ---------------- banded scores -> deinterleaved DRAM plane (fp16)
        sdma = [nc.sync, nc.scalar]
        for c in range(2):
            ssb = work.tile([128, SLD], f16, name="ssb", tag="ssb", bufs=2)
            for s in range(SUBHEADS):
                dil, ms = DIL_S[s], M_S[s]
                pscore = ps_mm.tile([128, ms], f32, name=f"psc{s}{c}", tag="mm")
                nc.tensor.matmul(pscore, qTs[s][:, c * 128 : c * 128 + 128],
                                 kTs[s][:, OFF_S[s] : OFF_S[s] + ms],
                                 start=True, stop=True)
                if dil == 1:
                    if c == 0:
                        nc.scalar.copy(ssb[:, SOFF[s] : SOFF[s] + ms], pscore)
                    else:
                        nc.vector.tensor_copy(ssb[:, SOFF[s] : SOFF[s] + ms], pscore)
                else:
                    # deinterleave m -> (m%dil, m//dil) during PSUM->SBUF copy
                    psrc = AP(pscore.tensor, pscore.offset,
                              [[ms, 128], [1, dil], [dil, ms // dil]])
                    dst = AP(ssb.tensor, ssb.offset + SOFF[s],
                             [[SLD, 128], [ms // dil, dil], [1, ms // dil]])
                    if c == 0:
                        nc.scalar.copy(dst, psrc)
                    else:
                        nc.vector.tensor_copy(dst, psrc)
                # per-subhead plane write for earlier downstream starts
                sdma[c].dma_start(
                    out=AP(splanes[s], c * 128 * ms, [[ms, 128], [1, ms]]),
                    in_=ssb[:, SOFF[s] : SOFF[s] + ms])

        # ---------------- V projection emit helper (interleaved into the
        # softmax gather-latency windows to keep the PE queue fed)
        # stage dil=8 vT columns into plane-col order (stationary APs are 2-dim only)
        vs8 = acts.tile([DH, SUBHEADS, 512], f16, name="vs8")
        o8 = PACK_OFF[5]
        for dc in range(SUBHEADS):
            nc.gpsimd.tensor_copy(
                AP(vs8.tensor, vs8.offset + dc * 512,
                   [[SUBHEADS * 512, DH], [128, 4], [64, 2], [1, 64]]),
                AP(vT.tensor, vT.offset + dc * KVP + o8,
                   [[SUBHEADS * KVP, DH], [2, 4], [1, 2], [8, 64]]),
            )

        vtiles = {}  # (pack, mc) -> [cw, len(g)*128] f16

        def emit_vproj(packs):
            for p in packs:
                g = PACKS[p]
                npk = len(g) * DH
                for mc, (cb, cw) in enumerate(PACK_CHUNKS[p]):
                    pv = ps_mm.tile([cw, npk], f32, name=f"pv{p}{mc}", tag="mm")
                    for dc in range(SUBHEADS):
                        nc.tensor.matmul(pv, _vchunk_ap(AP, vT, vs8, dc, p, cb, cw),
                                         Vp[p][:, dc, :],
                                         start=(dc == 0), stop=(dc == SUBHEADS - 1))
                    t = acts.tile([cw, npk], f16, name=f"v{p}_{mc}")
                    if (p + mc) % 2 == 0:
                        nc.vector.tensor_copy(t, pv)
                    else:
                        nc.scalar.copy(t, pv)
                    vtiles[(p, mc)] = t
                    if debug:
                        for gi, h in enumerate(g):
                            dv = work.tile([cw, DH], f32, name="dv", tag="dbg", bufs=2)
                            nc.vector.tensor_copy(dv, t[:, gi * DH : gi * DH + DH])
                            nc.sync.dma_start(
                                out=AP(dbg_vt, (WOFF[h] + cb) * DH, [[DH, cw], [1, DH]]),
                                in_=dv)

        # ---------------- band extract + Sk + softmax -> W (fp16)
        for c in range(2):
            bandTs = []
            for s in range(SUBHEADS):
                dil, ms = DIL_S[s], M_S[s]
                band = work.tile([128, KC], f16, name="band", tag="band", bufs=6)
                if dil == 1:
                    _g = sdma[c].dma_start(
                        out=band,
                        in_=AP(splanes[s], c * 128 * (ms + 1),
                               [[ms + 1, 128], [1, KC]]))
                    if c == 0 and s == 0:
                        _gath0 = _g
                else:
                    # one gather; band rows permuted rho-major:
                    # row rho*(128/dil)+P holds query q = dil*P + rho
                    sdma[c].dma_start(
                        out=band,
                        in_=AP(splanes[s],
                               c * 128 * ms + (c * 128) // dil,
                               [[ms + ms // dil, dil], [dil * ms + 1, 128 // dil], [1, KC]]))
                pbt = ps_sm.tile([KC, 128], f16, name="pbt", tag="sm")
                nc.tensor.transpose(pbt, band, identh)
                # row 32 = ones (pairs with the Sb row folded into SkT's last row)
                bt = work.tile([KC + 1, 128], f16, name="bt", tag="bt", bufs=5)
                nc.vector.memset(bt[KC : KC + 1], 1.0)
                if c == 0:
                    nc.scalar.copy(bt[:KC], pbt)
                else:
                    nc.vector.tensor_copy(bt[:KC], pbt)
                bandTs.append(bt)

            e = work.tile([128, HJ], f32, name="e", tag="e", bufs=2)
            psk = ps_mm.tile([128, HJ], f32, name="psk", tag="mm")
            hlo = 0
            for s in range(SUBHEADS):
                ncols = SUPER[s] * KC
                nc.tensor.matmul(psk[:, hlo : hlo + ncols], bandTs[s],
                                 SkT[:, hlo : hlo + ncols],
                                 start=True, stop=True, skip_group_check=True)
                hlo += ncols
            nc.scalar.activation(e, psk, mybir.ActivationFunctionType.Exp)
            z = work.tile([128, HEADS], f32, name="z", tag="z", bufs=4)
            nc.vector.reduce_sum(z, e.rearrange("p (h k) -> p h k", k=KC),
                                 axis=mybir.AxisListType.X)
            rz = work.tile([128, HEADS], f32, name="rz", tag="z", bufs=4)
            nc.vector.reciprocal(rz, z)
            w = work.tile([128, HJ], f16, name="w", tag="w", bufs=2)
            nc.vector.tensor_mul(
                w.rearrange("p (h k) -> p h k", k=KC),
                e.rearrange("p (h k) -> p h k", k=KC),
                AP(rz.tensor, rz.offset, [[HEADS, 128], [1, HEADS], [0, KC]]),
            )

            if debug:
                dw = work.tile([128, HJ], f32, name="dw", tag="dbg", bufs=2)
                nc.vector.tensor_copy(dw, w)
                nc.sync.dma_start(
                    out=AP(dbg_w, c * 128 * HJ, [[HJ, 128], [1, HJ]]), in_=dw)

            # ---- scatter W into the zeroed plane parts ([q, m] layout,
            # contiguous runs). Each part is its own DRAM tensor, so the six
            # scatters per chunk are fully independent.
            for half in range(2):
                wd = 1600
                sdma[half].dma_start(
                    out=AP(wparts[half], c * 128 * (wd + 1),
                           [[wd + 1, 128], [320, 5], [1, KC]]),
                    in_=AP(w.tensor, w.offset + half * 5 * KC,
                           [[HJ, 128], [KC, 5], [1, KC]]),
                )
            for h in range(10, HEADS):
                dil = DIL_H[h]
                wres = W_RES[dil]
                wd = 512
                base = c * 128 * wd + (c * 128) // dil
                sdma[h % 2].dma_start(
                    out=AP(wparts[PART_OF_H[h]], base,
                           [[wd + wres, dil], [dil * wd + 1, 128 // dil], [1, KC]]),
                    in_=AP(w.tensor, w.offset + h * KC, [[HJ, 128], [1, KC]]),
                )

        emit_vproj([0, 1, 2, 3, 4, 5])

        # ---------------- W plane bulk readback (per part, pipelined) + PV
        pcol = {0: 0, 1: 1600, 2: 3200, 3: 3712, 4: 4224, 5: 4736}
        wpl = []
        for c in range(2):
            t = acts.tile([128, WLD], f16, name=f"wpl{c}")
            # single-head parts first: their scatters drain fastest, letting
            # PV on heads 10-13 start while the 5-head scatters finish
            for pi, hs, wd in [WPARTS[i] for i in (2, 3, 4, 5, 0, 1)]:
                sdma[pi % 2].dma_start(
                    out=t[:, pcol[pi] : pcol[pi] + wd],
                    in_=AP(wparts[pi], c * 128 * wd, [[wd, 128], [1, wd]]))
            wpl.append(t)

        # collapse weights: load during the softmax window (DMA slack there)
        CkT = consts.tile([DH, HEADS, D], f16)   # f-chunk h on partitions' free dim
        _ck = nc.gpsimd.dma_start(
            out=CkT, in_=AP(CkT_d, 0, [[D, DH], [DH * D, HEADS], [1, D]]))
        add_dep_helper(_ck.ins, _gath0.ins, sync=True,
                       reason="defer collapse weights into the softmax DMA window")
        Cb = consts.tile([DH, D], f32)
        _cb = nc.gpsimd.dma_start(out=Cb, in_=AP(Cb_d, 0, [[0, DH], [1, D]]))
        add_dep_helper(_cb.ins, _gath0.ins, sync=True,
                       reason="defer collapse bias into the softmax DMA window")
        # PV per head pair: one merged W-transpose PSUM tile + one wft copy per
        # head, one paired PSUM drain for the attn outputs. Vb is folded into
        # the collapse bias on the host (softmax rows sum to 1), so the attn
        # drain is a plain copy.
        atiles = [None] * HEADS
        for pair_i, (ha, hb) in enumerate(
                [(10, 11), (12, 13), (0, 1), (2, 3), (4, 5), (6, 7), (8, 9)]):
            pat2 = ps_at.tile([DH, 2 * Q], f32, name=f"pat{ha}", tag="at")
            for hl, h in enumerate((ha, hb)):
                p, hh = PACK_OF_H[h]
                chunks = CHUNKS_H[h]
                nch = len(chunks)
                ptp = ps_sm.tile([128, 256 * nch], f16, name="ptp", tag="sm")
                for mc, (cb, cw) in enumerate(chunks):
                    for c in range(2):
                        nc.tensor.matmul(
                            ptp[:cw, mc * 256 + c * 128 : mc * 256 + c * 128 + 128],
                            wpl[c][:, WOFF[h] + cb : WOFF[h] + cb + cw],
                            identh, is_transpose=True, skip_group_check=True)
                wft = wftp.tile([128, 256 * nch], f16, name="wft", tag="wft")
                if h % 2 == 0:
                    nc.vector.tensor_copy(wft, ptp)
                else:
                    nc.scalar.copy(wft, ptp)
                for mc, (cb, cw) in enumerate(chunks):
                    nc.tensor.matmul(pat2[:, hl * Q : hl * Q + Q],
                                     vtiles[(p, mc)][:, hh * DH : hh * DH + DH],
                                     wft[:cw, mc * 256 : mc * 256 + 256],
                                     start=(mc == 0), stop=(mc == nch - 1),
                                     skip_group_check=True)
            at2 = actp.tile([DH, 2 * Q], f16, name=f"at{ha}", tag="at", bufs=7)
            if pair_i % 2 == 0:
                nc.vector.tensor_copy(at2, pat2)
            else:
                nc.scalar.copy(at2, pat2)
            atiles[ha] = at2[:, 0:Q]
            atiles[hb] = at2[:, Q : 2 * Q]
            if debug:
                for hl, h in enumerate((ha, hb)):
                    da = work.tile([DH, Q], f32, name="da", tag="dbg", bufs=2)
                    nc.vector.tensor_copy(da, at2[:, hl * Q : hl * Q + Q])
                    nc.sync.dma_start(
                        out=AP(dbg_at, h * DH * Q, [[Q, DH], [1, Q]]), in_=da)

        # ---------------- collapse + output
        outsb = [work.tile([128, D], f32, name=f"osb{c}", tag="osb", bufs=2) for c in range(2)]
        for c in range(2):
            for half in range(2):
                pc = ps_co.tile([128, 320], f32, name=f"pc{c}{half}", tag="co")
                for h in range(HEADS):
                    nc.tensor.matmul(pc, atiles[h][:, c * 128 : c * 128 + 128],
                                     CkT[:, h, half * 320 : half * 320 + 320],
                                     start=(h == 0), stop=(h == HEADS - 1))
                nc.vector.tensor_add(
                    outsb[c][:, half * 320 : half * 320 + 320], pc,
                    Cb[:, half * 320 : half * 320 + 320],
                )
            nc.sync.dma_start(
                out=AP(out_d, c * 128 * D, [[D, 128], [1, D]]),
                in_=outsb[c],
            )

    nc.finalize()
    return nc


def _prep_in_maps(inputs):
    f16 = np.float16
    query = np.asarray(inputs["query"], np.float32)
    key = np.asarray(inputs["key"], np.float32)
    value = np.asarray(inputs["value"], np.float32)
    Qk = np.asarray(inputs["Qk"], np.float32)
    Qb = np.asarray(inputs["Qb"], np.float32)
    Kk = np.asarray(inputs["Kk"], np.float32)
    Kb = np.asarray(inputs["Kb"], np.float32)
    Vk = np.asarray(inputs["Vk"], np.float32)
    Vb = np.asarray(inputs["Vb"], np.float32)
    Sk = np.asarray(inputs["Sk"], np.float32)
    Sb = np.asarray(inputs["Sb"], np.float32)
    Ck = np.asarray(inputs["Ck"], np.float32)
    Cb = np.asarray(inputs["Cb"], np.float32)

    QkT = np.ascontiguousarray(Qk.transpose(0, 2, 1)).astype(f16)  # [5, 640, 128]
    KkT = np.ascontiguousarray(Kk.transpose(0, 2, 1)).astype(f16)
    VkT = Vk.transpose(0, 2, 1)                                    # [14, 640, 128]
    Vp = [np.ascontiguousarray(
            np.concatenate([VkT[h] for h in g], axis=1)).astype(f16)
          for g in PACKS]
    SkT = np.concatenate(
        [Sk.transpose(2, 0, 1).reshape(KC, HJ), Sb.reshape(1, HJ)], axis=0
    ).astype(f16)                                                  # [33, 448]
    QbT = np.ascontiguousarray(Qb.T)                               # [128, 5]
    KbT = np.ascontiguousarray(Kb.T)
    CkT = np.ascontiguousarray(Ck.T).astype(f16)                   # [1792, 640]
    # Vb folded into the collapse bias: softmax rows sum to 1, so each head
    # contributes exactly Vb_h through the value path.
    Cb2 = Cb + Ck @ Vb.reshape(-1)
    Cbr = np.ascontiguousarray(Cb2.reshape(1, D))

    in_maps = []
    for c in range(NCORES):
        b, t0 = c // 4, (c % 4) * Q
        kpad = np.zeros((KV, D), np.float32)
        vpad = np.zeros((KV, D), np.float32)
        lo, hi = max(0, t0 - HALO), min(N, t0 + Q + 132)
        kpad[lo - (t0 - HALO) : hi - (t0 - HALO)] = key[b, lo:hi]
        vpad[lo - (t0 - HALO) : hi - (t0 - HALO)] = value[b, lo:hi]
        m = {
            "qT": np.ascontiguousarray(query[b, t0 : t0 + Q].T).astype(f16),
            "kT": np.ascontiguousarray(kpad.T).astype(f16),
            "vT": np.ascontiguousarray(vpad.T).astype(f16),
            "QkT": QkT, "KkT": KkT,
            "SkT": SkT, "QbT": QbT, "KbT": KbT,
            "CkT": CkT, "Cb": Cbr,
        }
        for p in range(len(PACKS)):
            m[f"VkT{p}"] = Vp[p]
        in_maps.append(m)
    return in_maps


def _run(inputs, trace=False, tmpdir=None):
    global _BUILT
    _inject_ntff_hook()
    from concourse.bass_utils import run_bass_kernel_spmd

    if _BUILT is None:
        _BUILT = _build()
    in_maps = _prep_in_maps(inputs)
    r = run_bass_kernel_spmd(_BUILT, in_maps, core_ids=list(range(NCORES)),
                             trace=trace, tmpdir=tmpdir)
    out = np.empty((B, N, D), np.float32)
    for c in range(NCORES):
        b, t0 = c // 4, (c % 4) * Q
        out[b, t0 : t0 + Q] = r.results[c]["out"]
    return out, r


def kernel(**inputs) -> np.ndarray:
    out, _ = _run(inputs, trace=False)
    return out
